# revision 24
# baseline (speedup 1.0000x reference)
"""BiLSTM kernel for Trainium2 (8 NeuronCores, SPMD data-parallel).

Problem: x [B=64, S=512, I=512], BiLSTM with H=512, gate order (f, g, i, o).
Returns (output [S, B, 2H], h_f [B, H], h_b [B, H]) matching the reference.

Sharding: direction x batch. Cores 0-3 run the forward direction on batch
quarters (16 rows each); cores 4-7 run the backward direction on batch
quarters, fed time-reversed x (SPMD: all cores run the identical program,
only the data differs; host flips the backward outputs back).

Per-core program (Tile framework):
  Phase 1: xw = x @ W + b for all 8192 tokens (S*16) as dense matmuls
           (M-tiles of 128 tokens), spilled to DRAM f32r; emission is
           interleaved into the step loop (lookahead 4 M-tiles) so its
           matmuls fill PE gaps instead of delaying early steps.
  Phase 2: 512 sequential LSTM steps. Per step, gate pre-activations
           accumulate in PSUM via f32r matmuls (lhsT = h^T [512,16] chunks,
           rhs = U [512, 2048]) in pair-interleaved order (f,g then o,i);
           xw_t is injected as a 5th accumulation matmul (lhsT = 16x16
           identity, rhs = xw_t), so ACT applies sigmoid/tanh straight from
           PSUM. Activated gates are PE-transposed into a [128, 64] layout
           (13 ns each, back-to-back), where the whole c/h algebra runs with
           full 128-partition utilization — and h_new in that layout IS the
           next step's matmul lhsT, so there are no h transposes or copies
           on the recurrence critical path. hs is written to DRAM in the
           transposed layout; the host untransposes.

All matmul operands are float32r: full fp32 storage, PE runs at bf16 speed
for moving-dim >= 256, measured matmul error ~1.6e-4 relative (better than
tf32). End-to-end output error vs the fp32 reference lands ~1e-3.
"""

from contextlib import ExitStack

import numpy as np

import concourse.bacc as bacc
import concourse.bass as bass
import concourse.mybir as mybir
import concourse.tile as tile
from concourse.bass_utils import run_bass_kernel_spmd
from concourse.masks import make_identity

F32 = mybir.dt.float32
F32R = mybir.dt.float32r

S = 512
I = 512
H = 512
B = 64
NCORES = 8
BL = 16        # batch rows per core
G = 4 * H      # 2048 gate columns, order f|g|i|o
KC = 4         # contraction chunks of 128
NB = 4         # gate banks of 512 columns
SPT = 128 // BL  # steps per phase-1 M-tile (8)

GATE_FUNCS = [
    mybir.ActivationFunctionType.Sigmoid,  # f
    mybir.ActivationFunctionType.Tanh,     # g
    mybir.ActivationFunctionType.Sigmoid,  # i
    mybir.ActivationFunctionType.Sigmoid,  # o
]

# Set to True by test.py to collect a profile; exec time lands in LAST_EXEC_NS.
TRACE = False
LAST_EXEC_NS = None
LAST_RESULTS = None


def _build(n_steps):
    nc = bacc.Bacc("TRN2", target_bir_lowering=False, debug=False)
    ntok = n_steps * BL
    NT = ntok // 128  # phase-1 M-tiles

    xT = nc.declare_dram_parameter("xT", [I, ntok], F32R, isOutput=False)
    U = nc.declare_dram_parameter("U", [H, G], F32R, isOutput=False)
    W = nc.declare_dram_parameter("W", [I, G], F32R, isOutput=False)
    bv = nc.declare_dram_parameter("b", [G], F32, isOutput=False)
    hs = nc.declare_dram_parameter("hs", [n_steps, 128, KC * BL], F32R, isOutput=True)

    with tile.TileContext(nc) as tc, ExitStack() as ctx:
        const = ctx.enter_context(tc.tile_pool(name="const", bufs=1))
        U_sb = const.tile([128, KC, G], F32R, name="U_sb")
        nc.sync.dma_start(out=U_sb, in_=U.ap().rearrange("(k p) g -> p k g", p=128))
        W_sb = const.tile([128, KC, G], F32R, name="W_sb")
        nc.sync.dma_start(out=W_sb, in_=W.ap().rearrange("(k p) g -> p k g", p=128))
        b_sb = const.tile([128, G], F32, name="b_sb")
        bap = bv.ap()
        b_bcast = bass.AP(
            tensor=bap.tensor, offset=bap.offset, ap=[[0, 128], list(bap.ap[0])]
        )
        nc.gpsimd.dma_start(out=b_sb, in_=b_bcast)
        ident = const.tile([BL, BL], F32, name="ident")
        make_identity(nc, ident)
        identr = const.tile([BL, BL], F32R, name="identr")
        nc.vector.tensor_copy(identr, ident)

        dram = ctx.enter_context(tc.tile_pool(name="dram", bufs=1, space="DRAM"))
        xw_d = [
            dram.tile([128, G], F32R, tag=f"xw{m}", name=f"xw{m}") for m in range(NT)
        ]

        # ---------- phase 1: xw = x @ W + b for all tokens ----------
        p1l = ctx.enter_context(tc.tile_pool(name="p1l", bufs=3))
        st = ctx.enter_context(tc.tile_pool(name="st", bufs=2))
        xwp = ctx.enter_context(tc.tile_pool(name="xwp", bufs=6))
        gp = ctx.enter_context(tc.tile_pool(name="gp", bufs=3, space="PSUM"))
        smp = ctx.enter_context(tc.tile_pool(name="smp", bufs=1, space="PSUM"))
        tpp = ctx.enter_context(tc.tile_pool(name="tpp", bufs=2, space="PSUM"))
        wk = ctx.enter_context(tc.tile_pool(name="wk", bufs=2))
        p1o = ctx.enter_context(tc.tile_pool(name="p1o", bufs=2))
        xTr = xT.ap().rearrange("(k p) t -> p k t", p=128)

        def emit_p1_tile(m):
            lhs = p1l.tile([128, KC, 128], F32R, tag="lhs", name=f"lhs{m}")
            nc.sync.dma_start(out=lhs, in_=xTr[:, :, m * 128 : (m + 1) * 128])
            xwb = p1o.tile([128, G], F32R, tag="xwb", name=f"xwb{m}")
            for n in range(NB):
                nsl = slice(n * 512, (n + 1) * 512)
                ps = smp.tile([128, 512], F32, tag="mix", name=f"p1ps{m}_{n}")
                for k in range(KC):
                    nc.tensor.matmul(
                        ps,
                        lhsT=lhs[:, k, :],
                        rhs=W_sb[:, k, nsl],
                        start=(k == 0),
                        stop=(k == KC - 1),
                    )
                nc.vector.tensor_tensor(
                    xwb[:, nsl], ps, b_sb[:, nsl], mybir.AluOpType.add
                )
            nc.sync.dma_start(out=xw_d[m], in_=xwb)

        P1_LOOKAHEAD = 4
        for m in range(min(P1_LOOKAHEAD, NT)):
            emit_p1_tile(m)

        # ---------- phase 2: recurrence ----------

        hT = st.tile([128, KC, BL], F32R, tag="hT", name="hT_init")
        zsc = wk.tile([128, KC * BL], F32, tag="zsc", name="zsc")
        nc.vector.memset(zsc, 0.0)
        nc.vector.tensor_copy(hT, zsc)
        cT = st.tile([128, KC, BL], F32, tag="c", name="cT_init")
        nc.vector.memset(cT, 0.0)

        for t in range(n_steps):
            m, r = divmod(t, SPT)
            if r == 0 and m + P1_LOOKAHEAD < NT:
                emit_p1_tile(m + P1_LOOKAHEAD)
            xwt = xwp.tile([BL, G], F32R, tag="xwt", name=f"xwt{t}")
            nc.sync.dma_start(out=xwt, in_=xw_d[m][r * BL : (r + 1) * BL, :])

            gbs = []
            for n in range(NB):
                gbs.append(gp.tile([BL, 512], F32, tag="g", name=f"gates{t}_{n}"))
            acts = wk.tile([BL, G], F32, tag="acts", name=f"acts{t}")
            # transposed activations, 2 PSUM banks: f,g,i in bank 0 and o in
            # bank 1 (gate n lives at cols GOFF[n]:GOFF[n]+64), so the c/h
            # algebra's reads never serialize against o's late transposes.
            tpa = tpp.tile([128, 1024], F32, tag="tpa", name=f"tpa{t}")
            GOFF = [0, 64, 128, 512]

            def bankpair(n0, n1):
                # interleave the two banks' K-chunk matmuls, close each bank
                # with its xw-inject matmul, activate it, then transpose the
                # activated gate into [128, KC*BL] while the pair partner
                # still streams on PE.
                for k in range(KC):
                    for n in (n0, n1):
                        nc.tensor.matmul(
                            gbs[n],
                            lhsT=hT[:, k, :],
                            rhs=U_sb[:, k, n * 512 : (n + 1) * 512],
                            start=(k == 0),
                            stop=False,
                            skip_group_check=True,
                        )
                for n in (n0, n1):
                    nc.tensor.matmul(
                        gbs[n],
                        lhsT=identr,
                        rhs=xwt[:, n * 512 : (n + 1) * 512],
                        start=False,
                        stop=True,
                        skip_group_check=True,
                    )
                    nc.scalar.activation(
                        acts[:, n * 512 : (n + 1) * 512], gbs[n], GATE_FUNCS[n]
                    )
                    for c in range(KC):
                        nc.tensor.transpose(
                            tpa[:, GOFF[n] + c * BL : GOFF[n] + (c + 1) * BL],
                            acts[:, n * 512 + c * 128 : n * 512 + (c + 1) * 128],
                            ident,
                        )

            bankpair(0, 1)  # f, g
            # f,g-dependent algebra overlaps the i,o matmul stream
            cf = wk.tile([128, KC * BL], F32, tag="cf", name=f"cf{t}")
            nc.vector.tensor_tensor(
                cf, cT.rearrange("p c b -> p (c b)"), tpa[:, 0 : KC * BL],
                mybir.AluOpType.mult,
            )
            gsb = wk.tile([128, KC * BL], F32, tag="gsb", name=f"gsb{t}")
            nc.vector.tensor_copy(gsb, tpa[:, 64 : 64 + KC * BL])
            bankpair(3, 2)  # o, i  (i last: o-act clears ACT before the tail)
            gi = wk.tile([128, KC * BL], F32, tag="gi", name=f"gi{t}")
            cTn = st.tile([128, KC, BL], F32, tag="c", name=f"c{t}")
            tch = wk.tile([128, KC * BL], F32, tag="tch", name=f"tch{t}")
            hTn = st.tile([128, KC, BL], F32R, tag="hT", name=f"hT{t}")
            cTnf = cTn.rearrange("p c b -> p (c b)")
            hTnf = hTn.rearrange("p c b -> p (c b)")
            nc.vector.tensor_tensor(
                gi, gsb, tpa[:, 128 : 128 + KC * BL], mybir.AluOpType.mult
            )
            nc.vector.tensor_tensor(cTnf, cf, gi, mybir.AluOpType.add)
            nc.scalar.activation(tch, cTnf, mybir.ActivationFunctionType.Tanh)
            nc.vector.tensor_tensor(
                hTnf, tpa[:, 512 : 512 + KC * BL], tch, mybir.AluOpType.mult
            )
            nc.sync.dma_start(out=hs.ap()[t], in_=hTnf)
            hT = hTn
            cT = cTn

    nc.compile()
    return nc


_CACHE = {}


def _get_nc(n_steps):
    if n_steps not in _CACHE:
        _CACHE[n_steps] = _build(n_steps)
    return _CACHE[n_steps]


def kernel(x, mask, U_fwd, W_fwd, b_fwd, U_bwd, W_bwd, b_bwd, n_steps=None):
    global LAST_EXEC_NS, LAST_RESULTS
    x = np.asarray(x, dtype=np.float32)
    mask = np.asarray(mask)
    if n_steps is None:
        n_steps = x.shape[1]
    nc = _get_nc(n_steps)

    packs = {}
    for d, (U_, W_, b_) in (
        ("f", (U_fwd, W_fwd, b_fwd)),
        ("b", (U_bwd, W_bwd, b_bwd)),
    ):
        U_ = np.asarray(U_, dtype=np.float32)
        W_ = np.asarray(W_, dtype=np.float32)
        b_ = np.asarray(b_, dtype=np.float32)
        packs[d] = (
            np.ascontiguousarray(np.concatenate([U_[g] for g in range(4)], axis=1)),
            np.ascontiguousarray(np.concatenate([W_[g] for g in range(4)], axis=1)),
            np.ascontiguousarray(np.concatenate([b_[g] for g in range(4)])),
        )

    in_maps = []
    for c in range(NCORES):
        is_bwd = c >= 4
        q = c % 4
        xc = x[q * BL : (q + 1) * BL, :n_steps]
        if is_bwd:
            xc = xc[:, ::-1]
        Up, Wp, bp = packs["b" if is_bwd else "f"]
        xTc = np.ascontiguousarray(xc.transpose(2, 1, 0).reshape(I, n_steps * BL))
        in_maps.append({"xT": xTc, "U": Up, "W": Wp, "b": bp})

    r = run_bass_kernel_spmd(nc, in_maps, list(range(NCORES)), trace=TRACE)
    LAST_EXEC_NS = r.exec_time_ns
    LAST_RESULTS = r
    res = r.results

    def unpack(hs_t):
        # hs_t [S, 128, KC*BL] transposed-layout: hs_t[s, p, c*BL+b] = h[s, b, c*128+p]
        S_ = hs_t.shape[0]
        return hs_t.reshape(S_, 128, KC, BL).transpose(0, 3, 2, 1).reshape(S_, BL, H)

    fwd = np.concatenate([unpack(res[c]["hs"]) for c in range(4)], axis=1)
    bwd = np.concatenate([unpack(res[c]["hs"]) for c in range(4, 8)], axis=1)[::-1]
    output = np.ascontiguousarray(
        np.concatenate([fwd, bwd], axis=-1), dtype=np.float32
    )

    lengths = mask[:, :n_steps].sum(axis=1).astype(np.int32)
    fwd_idx = np.clip(lengths - 1, 0, None)
    bidx = np.arange(x.shape[0])
    h_f = np.ascontiguousarray(fwd[fwd_idx, bidx])
    h_b = np.ascontiguousarray(bwd[0, bidx])
    return output, h_f, h_b


# revision 33
# speedup vs baseline: 1.1109x; 1.1109x over previous
"""BiLSTM kernel for Trainium2 (8 NeuronCores, SPMD data-parallel).

Problem: x [B=64, S=512, I=512], BiLSTM with H=512, gate order (f, g, i, o).
Returns (output [S, B, 2H], h_f [B, H], h_b [B, H]) matching the reference.

Sharding: direction x batch. Cores 0-3 run the forward direction on batch
quarters (16 rows each); cores 4-7 run the backward direction on batch
quarters, fed time-reversed x (SPMD: all cores run the identical program,
only the data differs; host flips the backward outputs back).

Per-core program (Tile framework):
  Phase 1: xw = x @ W + b for all 8192 tokens (S*16) as dense matmuls
           (M-tiles of 128 tokens), spilled to DRAM f32r; emission is
           interleaved into the step loop (lookahead 4 M-tiles) so its
           matmuls fill PE gaps instead of delaying early steps.
  Phase 2: 512 sequential LSTM steps. Per step, gate pre-activations
           accumulate in PSUM via f32r matmuls (lhsT = h^T [512,16] chunks,
           rhs = U [512, 2048]) in pair-interleaved order (f,g then o,i);
           xw_t is injected as a 5th accumulation matmul (lhsT = 16x16
           identity, rhs = xw_t), so ACT applies sigmoid/tanh straight from
           PSUM. Activated gates are PE-transposed into a [128, 64] layout
           (13 ns each, back-to-back), where the whole c/h algebra runs with
           full 128-partition utilization — and h_new in that layout IS the
           next step's matmul lhsT, so there are no h transposes or copies
           on the recurrence critical path. hs is written to DRAM in the
           transposed layout; the host untransposes.

All matmul operands are float32r: full fp32 storage, PE runs at bf16 speed
for moving-dim >= 256, measured matmul error ~1.6e-4 relative (better than
tf32). End-to-end output error vs the fp32 reference lands ~1e-3.
"""

from contextlib import ExitStack

import numpy as np

import concourse.bacc as bacc
import concourse.bass as bass
import concourse.mybir as mybir
import concourse.tile as tile
from concourse.bass_utils import run_bass_kernel_spmd
from concourse.masks import make_identity

F32 = mybir.dt.float32
F32R = mybir.dt.float32r

S = 512
I = 512
H = 512
B = 64
NCORES = 8
BL = 16        # batch rows per core
G = 4 * H      # 2048 gate columns, order f|g|i|o
KC = 4         # contraction chunks of 128
NB = 4         # gate banks of 512 columns
SPT = 128 // BL  # steps per phase-1 M-tile (8)

GATE_FUNCS = [
    mybir.ActivationFunctionType.Sigmoid,  # f
    mybir.ActivationFunctionType.Tanh,     # g
    mybir.ActivationFunctionType.Sigmoid,  # i
    mybir.ActivationFunctionType.Sigmoid,  # o
]

# Set to True by test.py to collect a profile; exec time lands in LAST_EXEC_NS.
TRACE = False
LAST_EXEC_NS = None
LAST_RESULTS = None


def _build(n_steps):
    nc = bacc.Bacc("TRN2", target_bir_lowering=False, debug=False)
    ntok = n_steps * BL
    NT = ntok // 128  # phase-1 M-tiles

    xT = nc.declare_dram_parameter("xT", [I, ntok], F32R, isOutput=False)
    U = nc.declare_dram_parameter("U", [H, G], F32R, isOutput=False)
    W = nc.declare_dram_parameter("W", [I, G], F32R, isOutput=False)
    bv = nc.declare_dram_parameter("b", [G], F32, isOutput=False)
    hs = nc.declare_dram_parameter("hs", [n_steps, 128, KC * BL], F32R, isOutput=True)

    with tile.TileContext(nc) as tc, ExitStack() as ctx:
        const = ctx.enter_context(tc.tile_pool(name="const", bufs=1))
        U_sb = const.tile([128, KC, G], F32R, name="U_sb")
        nc.sync.dma_start(out=U_sb, in_=U.ap().rearrange("(k p) g -> p k g", p=128))
        W_sb = const.tile([128, KC, G], F32R, name="W_sb")
        nc.sync.dma_start(out=W_sb, in_=W.ap().rearrange("(k p) g -> p k g", p=128))
        b_sb = const.tile([128, G], F32, name="b_sb")
        bap = bv.ap()
        b_bcast = bass.AP(
            tensor=bap.tensor, offset=bap.offset, ap=[[0, 128], list(bap.ap[0])]
        )
        nc.gpsimd.dma_start(out=b_sb, in_=b_bcast)
        ident = const.tile([BL, BL], F32, name="ident")
        make_identity(nc, ident)
        identr = const.tile([BL, BL], F32R, name="identr")
        nc.vector.tensor_copy(identr, ident)

        dram = ctx.enter_context(tc.tile_pool(name="dram", bufs=1, space="DRAM"))
        xw_d = [
            dram.tile([128, G], F32R, tag=f"xw{m}", name=f"xw{m}") for m in range(NT)
        ]

        # ---------- phase 1: xw = x @ W + b for all tokens ----------
        p1l = ctx.enter_context(tc.tile_pool(name="p1l", bufs=3))
        st = ctx.enter_context(tc.tile_pool(name="st", bufs=2))
        xwp = ctx.enter_context(tc.tile_pool(name="xwp", bufs=6))
        gp = ctx.enter_context(tc.tile_pool(name="gp", bufs=3, space="PSUM"))
        smp = ctx.enter_context(tc.tile_pool(name="smp", bufs=1, space="PSUM"))
        tpp = ctx.enter_context(tc.tile_pool(name="tpp", bufs=2, space="PSUM"))
        wk = ctx.enter_context(tc.tile_pool(name="wk", bufs=2))
        p1o = ctx.enter_context(tc.tile_pool(name="p1o", bufs=2))
        xTr = xT.ap().rearrange("(k p) t -> p k t", p=128)

        def emit_p1_tile(m):
            lhs = p1l.tile([128, KC, 128], F32R, tag="lhs", name=f"lhs{m}")
            nc.sync.dma_start(out=lhs, in_=xTr[:, :, m * 128 : (m + 1) * 128])
            xwb = p1o.tile([128, G], F32R, tag="xwb", name=f"xwb{m}")
            for n in range(NB):
                nsl = slice(n * 512, (n + 1) * 512)
                ps = smp.tile([128, 512], F32, tag="mix", name=f"p1ps{m}_{n}")
                for k in range(KC):
                    nc.tensor.matmul(
                        ps,
                        lhsT=lhs[:, k, :],
                        rhs=W_sb[:, k, nsl],
                        start=(k == 0),
                        stop=(k == KC - 1),
                    )
                nc.vector.tensor_tensor(
                    xwb[:, nsl], ps, b_sb[:, nsl], mybir.AluOpType.add
                )
            nc.sync.dma_start(out=xw_d[m], in_=xwb)

        P1_LOOKAHEAD = 4
        for m in range(min(P1_LOOKAHEAD, NT)):
            emit_p1_tile(m)

        # ---------- phase 2: recurrence ----------

        hT = st.tile([128, KC, BL], F32R, tag="hT", name="hT_init")
        zsc = wk.tile([128, KC * BL], F32, tag="zsc", name="zsc")
        nc.vector.memset(zsc, 0.0)
        nc.vector.tensor_copy(hT, zsc)
        cT = st.tile([128, KC, BL], F32, tag="c", name="cT_init")
        nc.vector.memset(cT, 0.0)

        for t in range(n_steps):
            m, r = divmod(t, SPT)
            if r == 0 and m + P1_LOOKAHEAD < NT:
                emit_p1_tile(m + P1_LOOKAHEAD)
            xwt = xwp.tile([BL, G], F32R, tag="xwt", name=f"xwt{t}")
            nc.sync.dma_start(out=xwt, in_=xw_d[m][r * BL : (r + 1) * BL, :])

            gbs = []
            for n in range(NB):
                gbs.append(gp.tile([BL, 512], F32, tag="g", name=f"gates{t}_{n}"))
            acts = wk.tile([BL, G], F32, tag="acts", name=f"acts{t}")
            # transposed activations, 2 PSUM banks: f,g,i in bank 0 and o in
            # bank 1 (gate n lives at cols GOFF[n]:GOFF[n]+64), so the c/h
            # algebra's reads never serialize against o's late transposes.
            tpa = tpp.tile([128, 512], F32, tag="tpa", name=f"tpa{t}")
            tpo = tpp.tile([128, KC * BL], F32, tag="tpo", name=f"tpo{t}")
            GOFF = [0, 64, 128, 0]

            def bankpair(n0, n1):
                # interleave the two banks' K-chunk matmuls, close each bank
                # with its xw-inject matmul, activate it, then transpose the
                # activated gate into [128, KC*BL] while the pair partner
                # still streams on PE.
                for k in range(KC - 1):
                    for n in (n0, n1):
                        nc.tensor.matmul(
                            gbs[n],
                            lhsT=hT[:, k, :],
                            rhs=U_sb[:, k, n * 512 : (n + 1) * 512],
                            start=(k == 0),
                            stop=False,
                            skip_group_check=True,
                        )
                act_insts = []
                for n in (n0, n1):
                    nc.tensor.matmul(
                        gbs[n],
                        lhsT=hT[:, KC - 1, :],
                        rhs=U_sb[:, KC - 1, n * 512 : (n + 1) * 512],
                        start=False,
                        stop=False,
                        skip_group_check=True,
                    )
                    nc.tensor.matmul(
                        gbs[n],
                        lhsT=identr,
                        rhs=xwt[:, n * 512 : (n + 1) * 512],
                        start=False,
                        stop=True,
                        skip_group_check=True,
                    )
                    ai = nc.scalar.activation(
                        acts[:, n * 512 : (n + 1) * 512], gbs[n], GATE_FUNCS[n]
                    )
                    act_insts.append(ai.ins)
                    dst = tpo if n == 3 else tpa
                    for c in range(KC):
                        nc.tensor.transpose(
                            dst[:, GOFF[n] + c * BL : GOFF[n] + (c + 1) * BL],
                            acts[:, n * 512 + c * 128 : n * 512 + (c + 1) * 128],
                            ident,
                        )
                # order-only hint: n1's activation must not preempt n0's on
                # ACT (n0 feeds the longer downstream chain)
                tile.add_dep_helper(
                    act_insts[1], act_insts[0], sync=False,
                    reason="gate act order: chain-feeding gate first",
                )

            bankpair(0, 1)  # f, g
            # f,g-dependent algebra overlaps the i,o matmul stream
            cf = wk.tile([128, KC * BL], F32, tag="cf", name=f"cf{t}")
            nc.vector.tensor_tensor(
                cf, cT.rearrange("p c b -> p (c b)"), tpa[:, 0 : KC * BL],
                mybir.AluOpType.mult,
            )
            gsb = wk.tile([128, KC * BL], F32, tag="gsb", name=f"gsb{t}")
            nc.vector.tensor_copy(gsb, tpa[:, 64 : 64 + KC * BL])
            bankpair(2, 3)  # i, o  (o last: i-act starts first and feeds the gi/cn/tanh chain while o-act runs)
            gi = wk.tile([128, KC * BL], F32, tag="gi", name=f"gi{t}")
            cTn = st.tile([128, KC, BL], F32, tag="c", name=f"c{t}")
            tch = wk.tile([128, KC * BL], F32, tag="tch", name=f"tch{t}")
            hTn = st.tile([128, KC, BL], F32R, tag="hT", name=f"hT{t}")
            cTnf = cTn.rearrange("p c b -> p (c b)")
            hTnf = hTn.rearrange("p c b -> p (c b)")
            nc.vector.tensor_tensor(
                gi, gsb, tpa[:, 128 : 128 + KC * BL], mybir.AluOpType.mult
            )
            nc.vector.tensor_tensor(cTnf, cf, gi, mybir.AluOpType.add)
            nc.scalar.activation(tch, cTnf, mybir.ActivationFunctionType.Tanh)
            nc.vector.tensor_tensor(
                hTnf, tpo[:, 0 : KC * BL], tch, mybir.AluOpType.mult
            )
            nc.sync.dma_start(out=hs.ap()[t], in_=hTnf)
            hT = hTn
            cT = cTn

    nc.compile()
    return nc


_CACHE = {}


def _get_nc(n_steps):
    if n_steps not in _CACHE:
        _CACHE[n_steps] = _build(n_steps)
    return _CACHE[n_steps]


def kernel(x, mask, U_fwd, W_fwd, b_fwd, U_bwd, W_bwd, b_bwd, n_steps=None):
    global LAST_EXEC_NS, LAST_RESULTS
    x = np.asarray(x, dtype=np.float32)
    mask = np.asarray(mask)
    if n_steps is None:
        n_steps = x.shape[1]
    nc = _get_nc(n_steps)

    packs = {}
    for d, (U_, W_, b_) in (
        ("f", (U_fwd, W_fwd, b_fwd)),
        ("b", (U_bwd, W_bwd, b_bwd)),
    ):
        U_ = np.asarray(U_, dtype=np.float32)
        W_ = np.asarray(W_, dtype=np.float32)
        b_ = np.asarray(b_, dtype=np.float32)
        packs[d] = (
            np.ascontiguousarray(np.concatenate([U_[g] for g in range(4)], axis=1)),
            np.ascontiguousarray(np.concatenate([W_[g] for g in range(4)], axis=1)),
            np.ascontiguousarray(np.concatenate([b_[g] for g in range(4)])),
        )

    in_maps = []
    for c in range(NCORES):
        is_bwd = c >= 4
        q = c % 4
        xc = x[q * BL : (q + 1) * BL, :n_steps]
        if is_bwd:
            xc = xc[:, ::-1]
        Up, Wp, bp = packs["b" if is_bwd else "f"]
        xTc = np.ascontiguousarray(xc.transpose(2, 1, 0).reshape(I, n_steps * BL))
        in_maps.append({"xT": xTc, "U": Up, "W": Wp, "b": bp})

    r = run_bass_kernel_spmd(nc, in_maps, list(range(NCORES)), trace=TRACE)
    LAST_EXEC_NS = r.exec_time_ns
    LAST_RESULTS = r
    res = r.results

    def unpack(hs_t):
        # hs_t [S, 128, KC*BL] transposed-layout: hs_t[s, p, c*BL+b] = h[s, b, c*128+p]
        S_ = hs_t.shape[0]
        return hs_t.reshape(S_, 128, KC, BL).transpose(0, 3, 2, 1).reshape(S_, BL, H)

    fwd = np.concatenate([unpack(res[c]["hs"]) for c in range(4)], axis=1)
    bwd = np.concatenate([unpack(res[c]["hs"]) for c in range(4, 8)], axis=1)[::-1]
    output = np.ascontiguousarray(
        np.concatenate([fwd, bwd], axis=-1), dtype=np.float32
    )

    lengths = mask[:, :n_steps].sum(axis=1).astype(np.int32)
    fwd_idx = np.clip(lengths - 1, 0, None)
    bidx = np.arange(x.shape[0])
    h_f = np.ascontiguousarray(fwd[fwd_idx, bidx])
    h_b = np.ascontiguousarray(bwd[0, bidx])
    return output, h_f, h_b


# revision 37
# speedup vs baseline: 1.2399x; 1.1161x over previous
"""BiLSTM kernel for Trainium2 (8 NeuronCores, SPMD data-parallel).

Problem: x [B=64, S=512, I=512], BiLSTM with H=512, gate order (f, g, i, o).
Returns (output [S, B, 2H], h_f [B, H], h_b [B, H]) matching the reference.

Sharding: direction x batch. Cores 0-3 run the forward direction on batch
quarters (16 rows each); cores 4-7 run the backward direction on batch
quarters, fed time-reversed x (SPMD: all cores run the identical program,
only the data differs; host flips the backward outputs back).

Per-core program (Tile framework):
  Phase 1: xw = x @ W + b for all 8192 tokens (S*16) as dense matmuls
           (M-tiles of 128 tokens), spilled to DRAM f32r; emission is
           interleaved into the step loop (lookahead 4 M-tiles) so its
           matmuls fill PE gaps instead of delaying early steps.
  Phase 2: 512 sequential LSTM steps. Per step, gate pre-activations
           accumulate in PSUM via f32r matmuls (lhsT = h^T [512,16] chunks,
           rhs = U [512, 2048]) in pair-interleaved order (f,g then o,i);
           xw_t is injected as a 5th accumulation matmul (lhsT = 16x16
           identity, rhs = xw_t), so ACT applies sigmoid/tanh straight from
           PSUM. Activated gates are PE-transposed into a [128, 64] layout
           (13 ns each, back-to-back), where the whole c/h algebra runs with
           full 128-partition utilization — and h_new in that layout IS the
           next step's matmul lhsT, so there are no h transposes or copies
           on the recurrence critical path. hs is written to DRAM in the
           transposed layout; the host untransposes.

All matmul operands are float32r: full fp32 storage, PE runs at bf16 speed
for moving-dim >= 256, measured matmul error ~1.6e-4 relative (better than
tf32). End-to-end output error vs the fp32 reference lands ~1e-3.
"""

from contextlib import ExitStack

import numpy as np

import concourse.bacc as bacc
import concourse.bass as bass
import concourse.mybir as mybir
import concourse.tile as tile
from concourse.bass_utils import run_bass_kernel_spmd
from concourse.masks import make_identity

F32 = mybir.dt.float32
F32R = mybir.dt.float32r

S = 512
I = 512
H = 512
B = 64
NCORES = 8
BL = 16        # batch rows per core
G = 4 * H      # 2048 gate columns, order f|g|i|o
KC = 4         # contraction chunks of 128
NB = 4         # gate banks of 512 columns
SPT = 128 // BL  # steps per phase-1 M-tile (8)

GATE_FUNCS = [
    mybir.ActivationFunctionType.Sigmoid,  # f
    mybir.ActivationFunctionType.Tanh,     # g
    mybir.ActivationFunctionType.Sigmoid,  # i
    mybir.ActivationFunctionType.Sigmoid,  # o
]

# Set to True by test.py to collect a profile; exec time lands in LAST_EXEC_NS.
TRACE = False
LAST_EXEC_NS = None
LAST_RESULTS = None


def _build(n_steps):
    nc = bacc.Bacc("TRN2", target_bir_lowering=False, debug=False)
    ntok = n_steps * BL
    NT = ntok // 128  # phase-1 M-tiles

    xT = nc.declare_dram_parameter("xT", [I, ntok], F32R, isOutput=False)
    U = nc.declare_dram_parameter("U", [H, G], F32R, isOutput=False)
    W = nc.declare_dram_parameter("W", [I, G], F32R, isOutput=False)
    bv = nc.declare_dram_parameter("b", [G], F32, isOutput=False)
    hs = nc.declare_dram_parameter("hs", [n_steps, 128, KC * BL], F32R, isOutput=True)

    with tile.TileContext(nc) as tc, ExitStack() as ctx:
        const = ctx.enter_context(tc.tile_pool(name="const", bufs=1))
        U_sb = const.tile([128, KC, G], F32R, name="U_sb")
        nc.sync.dma_start(out=U_sb, in_=U.ap().rearrange("(k p) g -> p k g", p=128))
        W_sb = const.tile([128, KC, G], F32R, name="W_sb")
        nc.sync.dma_start(out=W_sb, in_=W.ap().rearrange("(k p) g -> p k g", p=128))
        b_sb = const.tile([128, G], F32, name="b_sb")
        bap = bv.ap()
        b_bcast = bass.AP(
            tensor=bap.tensor, offset=bap.offset, ap=[[0, 128], list(bap.ap[0])]
        )
        nc.gpsimd.dma_start(out=b_sb, in_=b_bcast)
        ident = const.tile([BL, BL], F32, name="ident")
        make_identity(nc, ident)
        identr = const.tile([BL, BL], F32R, name="identr")
        nc.vector.tensor_copy(identr, ident)

        dram = ctx.enter_context(tc.tile_pool(name="dram", bufs=1, space="DRAM"))
        xw_d = [
            dram.tile([128, G], F32R, tag=f"xw{m}", name=f"xw{m}") for m in range(NT)
        ]

        # ---------- phase 1: xw = x @ W + b for all tokens ----------
        p1l = ctx.enter_context(tc.tile_pool(name="p1l", bufs=3))
        st = ctx.enter_context(tc.tile_pool(name="st", bufs=2))
        xwp = ctx.enter_context(tc.tile_pool(name="xwp", bufs=6))
        gp = ctx.enter_context(tc.tile_pool(name="gp", bufs=5, space="PSUM"))
        smp = ctx.enter_context(tc.tile_pool(name="smp", bufs=1, space="PSUM"))
        tpp = ctx.enter_context(tc.tile_pool(name="tpp", bufs=1, space="PSUM"))
        wk = ctx.enter_context(tc.tile_pool(name="wk", bufs=2))
        p1o = ctx.enter_context(tc.tile_pool(name="p1o", bufs=2))
        xTr = xT.ap().rearrange("(k p) t -> p k t", p=128)

        def emit_p1_tile(m):
            lhs = p1l.tile([128, KC, 128], F32R, tag="lhs", name=f"lhs{m}")
            nc.sync.dma_start(out=lhs, in_=xTr[:, :, m * 128 : (m + 1) * 128])
            xwb = p1o.tile([128, G], F32R, tag="xwb", name=f"xwb{m}")
            for n in range(NB):
                nsl = slice(n * 512, (n + 1) * 512)
                ps = smp.tile([128, 512], F32, tag="mix", name=f"p1ps{m}_{n}")
                for k in range(KC):
                    nc.tensor.matmul(
                        ps,
                        lhsT=lhs[:, k, :],
                        rhs=W_sb[:, k, nsl],
                        start=(k == 0),
                        stop=(k == KC - 1),
                    )
                nc.vector.tensor_tensor(
                    xwb[:, nsl], ps, b_sb[:, nsl], mybir.AluOpType.add
                )
            nc.sync.dma_start(out=xw_d[m], in_=xwb)

        P1_LOOKAHEAD = 4
        for m in range(min(P1_LOOKAHEAD, NT)):
            emit_p1_tile(m)

        # ---------- phase 2: recurrence ----------

        hT = st.tile([128, KC, BL], F32R, tag="hT", name="hT_init")
        zsc = wk.tile([128, KC * BL], F32, tag="zsc", name="zsc")
        nc.vector.memset(zsc, 0.0)
        nc.vector.tensor_copy(hT, zsc)
        cT = st.tile([128, KC, BL], F32, tag="c", name="cT_init")
        nc.vector.memset(cT, 0.0)

        for t in range(n_steps):
            m, r = divmod(t, SPT)
            if r == 0 and m + P1_LOOKAHEAD < NT:
                emit_p1_tile(m + P1_LOOKAHEAD)
            xwt = xwp.tile([BL, G], F32R, tag="xwt", name=f"xwt{t}")
            nc.sync.dma_start(out=xwt, in_=xw_d[m][r * BL : (r + 1) * BL, :])

            gbs = []
            for n in range(NB):
                gbs.append(gp.tile([BL, 512], F32, tag="g", name=f"gates{t}_{n}"))
            acts = wk.tile([BL, G], F32, tag="acts", name=f"acts{t}")
            # transposed activations, 2 PSUM banks: f,g,i in bank 0 and o in
            # bank 1 (gate n lives at cols GOFF[n]:GOFF[n]+64), so the c/h
            # algebra's reads never serialize against o's late transposes.
            tpa = tpp.tile([128, 512], F32, tag="tpa", name=f"tpa{t}")
            tpo = tpp.tile([128, KC * BL], F32, tag="tpo", name=f"tpo{t}")
            GOFF = [0, 64, 128, 0]

            def bankpair(n0, n1):
                # interleave the two banks' K-chunk matmuls, close each bank
                # with its xw-inject matmul, activate it, then transpose the
                # activated gate into [128, KC*BL] while the pair partner
                # still streams on PE.
                # xw-inject matmuls OPEN each accumulation group: they only
                # need xwt (prefetched), so the scheduler hoists them into the
                # previous step's tail while PE is otherwise idle.
                for n in (n0, n1):
                    nc.tensor.matmul(
                        gbs[n],
                        lhsT=identr,
                        rhs=xwt[:, n * 512 : (n + 1) * 512],
                        start=True,
                        stop=False,
                        skip_group_check=True,
                    )
                for k in range(KC - 1):
                    for n in (n0, n1):
                        nc.tensor.matmul(
                            gbs[n],
                            lhsT=hT[:, k, :],
                            rhs=U_sb[:, k, n * 512 : (n + 1) * 512],
                            start=False,
                            stop=False,
                            skip_group_check=True,
                        )
                act_insts = []
                for n in (n0, n1):
                    nc.tensor.matmul(
                        gbs[n],
                        lhsT=hT[:, KC - 1, :],
                        rhs=U_sb[:, KC - 1, n * 512 : (n + 1) * 512],
                        start=False,
                        stop=True,
                        skip_group_check=True,
                    )
                    ai = nc.scalar.activation(
                        acts[:, n * 512 : (n + 1) * 512], gbs[n], GATE_FUNCS[n]
                    )
                    act_insts.append(ai.ins)
                    dst = tpo if n == 3 else tpa
                    for c in range(KC):
                        nc.tensor.transpose(
                            dst[:, GOFF[n] + c * BL : GOFF[n] + (c + 1) * BL],
                            acts[:, n * 512 + c * 128 : n * 512 + (c + 1) * 128],
                            ident,
                        )
                # order-only hint: n1's activation must not preempt n0's on
                # ACT (n0 feeds the longer downstream chain)
                tile.add_dep_helper(
                    act_insts[1], act_insts[0], sync=False,
                    reason="gate act order: chain-feeding gate first",
                )

            bankpair(0, 1)  # f, g
            # f,g-dependent algebra overlaps the i,o matmul stream
            cf = wk.tile([128, KC * BL], F32, tag="cf", name=f"cf{t}")
            nc.vector.tensor_tensor(
                cf, cT.rearrange("p c b -> p (c b)"), tpa[:, 0 : KC * BL],
                mybir.AluOpType.mult,
            )
            gsb = wk.tile([128, KC * BL], F32, tag="gsb", name=f"gsb{t}")
            nc.vector.tensor_copy(gsb, tpa[:, 64 : 64 + KC * BL])
            bankpair(2, 3)  # i, o  (o last: i-act starts first and feeds the gi/cn/tanh chain while o-act runs)
            gi = wk.tile([128, KC * BL], F32, tag="gi", name=f"gi{t}")
            cTn = st.tile([128, KC, BL], F32, tag="c", name=f"c{t}")
            tch = wk.tile([128, KC * BL], F32, tag="tch", name=f"tch{t}")
            hTn = st.tile([128, KC, BL], F32R, tag="hT", name=f"hT{t}")
            cTnf = cTn.rearrange("p c b -> p (c b)")
            hTnf = hTn.rearrange("p c b -> p (c b)")
            nc.vector.tensor_tensor(
                gi, gsb, tpa[:, 128 : 128 + KC * BL], mybir.AluOpType.mult
            )
            nc.vector.tensor_tensor(cTnf, cf, gi, mybir.AluOpType.add)
            nc.scalar.activation(tch, cTnf, mybir.ActivationFunctionType.Tanh)
            nc.vector.tensor_tensor(
                hTnf, tpo[:, 0 : KC * BL], tch, mybir.AluOpType.mult
            )
            nc.sync.dma_start(out=hs.ap()[t], in_=hTnf)
            hT = hTn
            cT = cTn

    nc.compile()
    return nc


_CACHE = {}


def _get_nc(n_steps):
    if n_steps not in _CACHE:
        _CACHE[n_steps] = _build(n_steps)
    return _CACHE[n_steps]


def kernel(x, mask, U_fwd, W_fwd, b_fwd, U_bwd, W_bwd, b_bwd, n_steps=None):
    global LAST_EXEC_NS, LAST_RESULTS
    x = np.asarray(x, dtype=np.float32)
    mask = np.asarray(mask)
    if n_steps is None:
        n_steps = x.shape[1]
    nc = _get_nc(n_steps)

    packs = {}
    for d, (U_, W_, b_) in (
        ("f", (U_fwd, W_fwd, b_fwd)),
        ("b", (U_bwd, W_bwd, b_bwd)),
    ):
        U_ = np.asarray(U_, dtype=np.float32)
        W_ = np.asarray(W_, dtype=np.float32)
        b_ = np.asarray(b_, dtype=np.float32)
        packs[d] = (
            np.ascontiguousarray(np.concatenate([U_[g] for g in range(4)], axis=1)),
            np.ascontiguousarray(np.concatenate([W_[g] for g in range(4)], axis=1)),
            np.ascontiguousarray(np.concatenate([b_[g] for g in range(4)])),
        )

    in_maps = []
    for c in range(NCORES):
        is_bwd = c >= 4
        q = c % 4
        xc = x[q * BL : (q + 1) * BL, :n_steps]
        if is_bwd:
            xc = xc[:, ::-1]
        Up, Wp, bp = packs["b" if is_bwd else "f"]
        xTc = np.ascontiguousarray(xc.transpose(2, 1, 0).reshape(I, n_steps * BL))
        in_maps.append({"xT": xTc, "U": Up, "W": Wp, "b": bp})

    r = run_bass_kernel_spmd(nc, in_maps, list(range(NCORES)), trace=TRACE)
    LAST_EXEC_NS = r.exec_time_ns
    LAST_RESULTS = r
    res = r.results

    def unpack(hs_t):
        # hs_t [S, 128, KC*BL] transposed-layout: hs_t[s, p, c*BL+b] = h[s, b, c*128+p]
        S_ = hs_t.shape[0]
        return hs_t.reshape(S_, 128, KC, BL).transpose(0, 3, 2, 1).reshape(S_, BL, H)

    fwd = np.concatenate([unpack(res[c]["hs"]) for c in range(4)], axis=1)
    bwd = np.concatenate([unpack(res[c]["hs"]) for c in range(4, 8)], axis=1)[::-1]
    output = np.ascontiguousarray(
        np.concatenate([fwd, bwd], axis=-1), dtype=np.float32
    )

    lengths = mask[:, :n_steps].sum(axis=1).astype(np.int32)
    fwd_idx = np.clip(lengths - 1, 0, None)
    bidx = np.arange(x.shape[0])
    h_f = np.ascontiguousarray(fwd[fwd_idx, bidx])
    h_b = np.ascontiguousarray(bwd[0, bidx])
    return output, h_f, h_b


# revision 40
# speedup vs baseline: 1.3503x; 1.0890x over previous
"""BiLSTM kernel for Trainium2 (8 NeuronCores, SPMD data-parallel).

Problem: x [B=64, S=512, I=512], BiLSTM with H=512, gate order (f, g, i, o).
Returns (output [S, B, 2H], h_f [B, H], h_b [B, H]) matching the reference.

Sharding: direction x batch. Cores 0-3 run the forward direction on batch
quarters (16 rows each); cores 4-7 run the backward direction on batch
quarters, fed time-reversed x (SPMD: all cores run the identical program,
only the data differs; host flips the backward outputs back).

Per-core program (Tile framework):
  Phase 1: xw = x @ W + b for all 8192 tokens (S*16) as dense matmuls
           (M-tiles of 128 tokens), spilled to DRAM f32r; emission is
           interleaved into the step loop (lookahead 4 M-tiles) so its
           matmuls fill PE gaps instead of delaying early steps.
  Phase 2: 512 sequential LSTM steps. Per step, gate pre-activations
           accumulate in PSUM via f32r matmuls (lhsT = h^T [512,16] chunks,
           rhs = U [512, 2048]) in pair-interleaved order (f,g then o,i);
           xw_t is injected as a 5th accumulation matmul (lhsT = 16x16
           identity, rhs = xw_t), so ACT applies sigmoid/tanh straight from
           PSUM. Activated gates are PE-transposed into a [128, 64] layout
           (13 ns each, back-to-back), where the whole c/h algebra runs with
           full 128-partition utilization — and h_new in that layout IS the
           next step's matmul lhsT, so there are no h transposes or copies
           on the recurrence critical path. hs is written to DRAM in the
           transposed layout; the host untransposes.

All matmul operands are float32r: full fp32 storage, PE runs at bf16 speed
for moving-dim >= 256, measured matmul error ~1.6e-4 relative (better than
tf32). End-to-end output error vs the fp32 reference lands ~1e-3.
"""

from contextlib import ExitStack

import numpy as np

import concourse.bacc as bacc
import concourse.bass as bass
import concourse.mybir as mybir
import concourse.tile as tile
from concourse.bass_utils import run_bass_kernel_spmd
from concourse.masks import make_identity

F32 = mybir.dt.float32
F32R = mybir.dt.float32r

S = 512
I = 512
H = 512
B = 64
NCORES = 8
BL = 16        # batch rows per core
G = 4 * H      # 2048 gate columns, order f|g|i|o
KC = 4         # contraction chunks of 128
NB = 4         # gate banks of 512 columns
SPT = 128 // BL  # steps per phase-1 M-tile (8)

GATE_FUNCS = [
    mybir.ActivationFunctionType.Sigmoid,  # f
    mybir.ActivationFunctionType.Tanh,     # g
    mybir.ActivationFunctionType.Sigmoid,  # i
    mybir.ActivationFunctionType.Sigmoid,  # o
]

# Set to True by test.py to collect a profile; exec time lands in LAST_EXEC_NS.
TRACE = False
LAST_EXEC_NS = None
LAST_RESULTS = None


def _build(n_steps):
    nc = bacc.Bacc("TRN2", target_bir_lowering=False, debug=False)
    ntok = n_steps * BL
    NT = ntok // 128  # phase-1 M-tiles

    xT = nc.declare_dram_parameter("xT", [I, ntok], F32R, isOutput=False)
    U = nc.declare_dram_parameter("U", [H, G], F32R, isOutput=False)
    W = nc.declare_dram_parameter("W", [I, G], F32R, isOutput=False)
    bv = nc.declare_dram_parameter("b", [G], F32, isOutput=False)
    hs = nc.declare_dram_parameter("hs", [n_steps, 128, KC * BL], F32R, isOutput=True)

    with tile.TileContext(nc) as tc, ExitStack() as ctx:
        const = ctx.enter_context(tc.tile_pool(name="const", bufs=1))
        U_sb = const.tile([128, KC, G], F32R, name="U_sb")
        nc.sync.dma_start(out=U_sb, in_=U.ap().rearrange("(k p) g -> p k g", p=128))
        W_sb = const.tile([128, KC, G], F32R, name="W_sb")
        nc.sync.dma_start(out=W_sb, in_=W.ap().rearrange("(k p) g -> p k g", p=128))
        b_sb = const.tile([128, G], F32, name="b_sb")
        bap = bv.ap()
        b_bcast = bass.AP(
            tensor=bap.tensor, offset=bap.offset, ap=[[0, 128], list(bap.ap[0])]
        )
        nc.gpsimd.dma_start(out=b_sb, in_=b_bcast)
        ident = const.tile([BL, BL], F32, name="ident")
        make_identity(nc, ident)
        identr = const.tile([BL, BL], F32R, name="identr")
        nc.vector.tensor_copy(identr, ident)

        dram = ctx.enter_context(tc.tile_pool(name="dram", bufs=1, space="DRAM"))
        xw_d = [
            dram.tile([128, G], F32R, tag=f"xw{m}", name=f"xw{m}") for m in range(NT)
        ]

        # ---------- phase 1: xw = x @ W + b for all tokens ----------
        p1l = ctx.enter_context(tc.tile_pool(name="p1l", bufs=3))
        st = ctx.enter_context(tc.tile_pool(name="st", bufs=2))
        xwp = ctx.enter_context(tc.tile_pool(name="xwp", bufs=6))
        gp = ctx.enter_context(tc.tile_pool(name="gp", bufs=5, space="PSUM"))
        smp = ctx.enter_context(tc.tile_pool(name="smp", bufs=1, space="PSUM"))
        tpp = ctx.enter_context(tc.tile_pool(name="tpp", bufs=1, space="PSUM"))
        wk = ctx.enter_context(tc.tile_pool(name="wk", bufs=2))
        p1o = ctx.enter_context(tc.tile_pool(name="p1o", bufs=2))
        xTr = xT.ap().rearrange("(k p) t -> p k t", p=128)

        def emit_p1_tile(m):
            lhs = p1l.tile([128, KC, 128], F32R, tag="lhs", name=f"lhs{m}")
            nc.sync.dma_start(out=lhs, in_=xTr[:, :, m * 128 : (m + 1) * 128])
            xwb = p1o.tile([128, G], F32R, tag="xwb", name=f"xwb{m}")
            for n in range(NB):
                nsl = slice(n * 512, (n + 1) * 512)
                ps = smp.tile([128, 512], F32, tag="mix", name=f"p1ps{m}_{n}")
                for k in range(KC):
                    nc.tensor.matmul(
                        ps,
                        lhsT=lhs[:, k, :],
                        rhs=W_sb[:, k, nsl],
                        start=(k == 0),
                        stop=(k == KC - 1),
                    )
                nc.vector.tensor_tensor(
                    xwb[:, nsl], ps, b_sb[:, nsl], mybir.AluOpType.add
                )
            nc.sync.dma_start(out=xw_d[m], in_=xwb)

        P1_LOOKAHEAD = 4
        for m in range(min(P1_LOOKAHEAD, NT)):
            emit_p1_tile(m)

        # ---------- phase 2: recurrence ----------

        hT = st.tile([128, KC, BL], F32R, tag="hT", name="hT_init")
        zsc = wk.tile([128, KC * BL], F32, tag="zsc", name="zsc")
        nc.vector.memset(zsc, 0.0)
        nc.vector.tensor_copy(hT, zsc)
        cT = st.tile([128, KC, BL], F32, tag="c", name="cT_init")
        nc.vector.memset(cT, 0.0)

        for t in range(n_steps):
            m, r = divmod(t, SPT)
            if r == 0 and m + P1_LOOKAHEAD < NT:
                emit_p1_tile(m + P1_LOOKAHEAD)
            xwt = xwp.tile([BL, G], F32R, tag="xwt", name=f"xwt{t}")
            nc.sync.dma_start(out=xwt, in_=xw_d[m][r * BL : (r + 1) * BL, :])

            gbs = []
            for n in range(NB):
                gbs.append(gp.tile([BL, 512], F32, tag="g", name=f"gates{t}_{n}"))
            acts = wk.tile([BL, G], F32, tag="acts", name=f"acts{t}")
            # transposed activations, 2 PSUM banks: f,g,i in bank 0 and o in
            # bank 1 (gate n lives at cols GOFF[n]:GOFF[n]+64), so the c/h
            # algebra's reads never serialize against o's late transposes.
            tpa = tpp.tile([128, 512], F32, tag="tpa", name=f"tpa{t}")
            tpo = tpp.tile([128, KC * BL], F32, tag="tpo", name=f"tpo{t}")
            GOFF = [0, 64, 128, 0]

            # xw-inject matmuls OPEN every accumulation group: they depend
            # only on the prefetched xw tile, so the scheduler hoists them
            # (and the p1 matmuls) into the previous step's tail while PE
            # would otherwise idle waiting for h.
            for n in (2, 1, 0, 3):
                nc.tensor.matmul(
                    gbs[n],
                    lhsT=identr,
                    rhs=xwt[:, n * 512 : (n + 1) * 512],
                    start=True,
                    stop=False,
                    skip_group_check=True,
                )
            # Bank-major K-chunks in order i, f, g, o: h arrives whole (one
            # hn write), so the chain-feeding banks can close as early as
            # possible and the tail chain (i/f/g acts -> gi/cf/cn -> tanh ->
            # hn) runs concurrently with the remaining o stream.
            act_insts = []
            for n in (2, 1, 0, 3):
                for k in range(KC):
                    nc.tensor.matmul(
                        gbs[n],
                        lhsT=hT[:, k, :],
                        rhs=U_sb[:, k, n * 512 : (n + 1) * 512],
                        start=False,
                        stop=(k == KC - 1),
                        skip_group_check=True,
                    )
                ai = nc.scalar.activation(
                    acts[:, n * 512 : (n + 1) * 512], gbs[n], GATE_FUNCS[n]
                )
                act_insts.append(ai.ins)
                dst = tpo if n == 3 else tpa
                for cc in range(KC):
                    nc.tensor.transpose(
                        dst[:, GOFF[n] + cc * BL : GOFF[n] + (cc + 1) * BL],
                        acts[:, n * 512 + cc * 128 : n * 512 + (cc + 1) * 128],
                        ident,
                    )
                if n == 0:
                    cf = wk.tile([128, KC * BL], F32, tag="cf", name=f"cf{t}")
                    nc.vector.tensor_tensor(
                        cf, cT.rearrange("p c b -> p (c b)"), tpa[:, 0 : KC * BL],
                        mybir.AluOpType.mult,
                    )
                if n == 1:
                    gsb = wk.tile([128, KC * BL], F32, tag="gsb", name=f"gsb{t}")
                    nc.vector.tensor_copy(gsb, tpa[:, 64 : 64 + KC * BL])
                    # emitted before f's transposes so the PSUM-tile dep
                    # tracker orders gi's read ahead of f's writes
                    gi = wk.tile([128, KC * BL], F32, tag="gi", name=f"gi{t}")
                    nc.vector.tensor_tensor(
                        gi, gsb, tpa[:, 128 : 128 + KC * BL], mybir.AluOpType.mult
                    )
            cTn = st.tile([128, KC, BL], F32, tag="c", name=f"c{t}")
            tch = wk.tile([128, KC * BL], F32, tag="tch", name=f"tch{t}")
            hTn = st.tile([128, KC, BL], F32R, tag="hT", name=f"hT{t}")
            cTnf = cTn.rearrange("p c b -> p (c b)")
            hTnf = hTn.rearrange("p c b -> p (c b)")
            nc.vector.tensor_tensor(cTnf, cf, gi, mybir.AluOpType.add)
            nc.scalar.activation(tch, cTnf, mybir.ActivationFunctionType.Tanh)
            nc.vector.tensor_tensor(
                hTnf, tpo[:, 0 : KC * BL], tch, mybir.AluOpType.mult
            )
            nc.sync.dma_start(out=hs.ap()[t], in_=hTnf)
            hT = hTn
            cT = cTn

    nc.compile()
    return nc


_CACHE = {}


def _get_nc(n_steps):
    if n_steps not in _CACHE:
        _CACHE[n_steps] = _build(n_steps)
    return _CACHE[n_steps]


def kernel(x, mask, U_fwd, W_fwd, b_fwd, U_bwd, W_bwd, b_bwd, n_steps=None):
    global LAST_EXEC_NS, LAST_RESULTS
    x = np.asarray(x, dtype=np.float32)
    mask = np.asarray(mask)
    if n_steps is None:
        n_steps = x.shape[1]
    nc = _get_nc(n_steps)

    packs = {}
    for d, (U_, W_, b_) in (
        ("f", (U_fwd, W_fwd, b_fwd)),
        ("b", (U_bwd, W_bwd, b_bwd)),
    ):
        U_ = np.asarray(U_, dtype=np.float32)
        W_ = np.asarray(W_, dtype=np.float32)
        b_ = np.asarray(b_, dtype=np.float32)
        packs[d] = (
            np.ascontiguousarray(np.concatenate([U_[g] for g in range(4)], axis=1)),
            np.ascontiguousarray(np.concatenate([W_[g] for g in range(4)], axis=1)),
            np.ascontiguousarray(np.concatenate([b_[g] for g in range(4)])),
        )

    in_maps = []
    for c in range(NCORES):
        is_bwd = c >= 4
        q = c % 4
        xc = x[q * BL : (q + 1) * BL, :n_steps]
        if is_bwd:
            xc = xc[:, ::-1]
        Up, Wp, bp = packs["b" if is_bwd else "f"]
        xTc = np.ascontiguousarray(xc.transpose(2, 1, 0).reshape(I, n_steps * BL))
        in_maps.append({"xT": xTc, "U": Up, "W": Wp, "b": bp})

    r = run_bass_kernel_spmd(nc, in_maps, list(range(NCORES)), trace=TRACE)
    LAST_EXEC_NS = r.exec_time_ns
    LAST_RESULTS = r
    res = r.results

    def unpack(hs_t):
        # hs_t [S, 128, KC*BL] transposed-layout: hs_t[s, p, c*BL+b] = h[s, b, c*128+p]
        S_ = hs_t.shape[0]
        return hs_t.reshape(S_, 128, KC, BL).transpose(0, 3, 2, 1).reshape(S_, BL, H)

    fwd = np.concatenate([unpack(res[c]["hs"]) for c in range(4)], axis=1)
    bwd = np.concatenate([unpack(res[c]["hs"]) for c in range(4, 8)], axis=1)[::-1]
    output = np.ascontiguousarray(
        np.concatenate([fwd, bwd], axis=-1), dtype=np.float32
    )

    lengths = mask[:, :n_steps].sum(axis=1).astype(np.int32)
    fwd_idx = np.clip(lengths - 1, 0, None)
    bidx = np.arange(x.shape[0])
    h_f = np.ascontiguousarray(fwd[fwd_idx, bidx])
    h_b = np.ascontiguousarray(bwd[0, bidx])
    return output, h_f, h_b


# revision 42
# speedup vs baseline: 1.3700x; 1.0145x over previous
"""BiLSTM kernel for Trainium2 (8 NeuronCores, SPMD data-parallel).

Problem: x [B=64, S=512, I=512], BiLSTM with H=512, gate order (f, g, i, o).
Returns (output [S, B, 2H], h_f [B, H], h_b [B, H]) matching the reference.

Sharding: direction x batch. Cores 0-3 run the forward direction on batch
quarters (16 rows each); cores 4-7 run the backward direction on batch
quarters, fed time-reversed x (SPMD: all cores run the identical program,
only the data differs; host flips the backward outputs back).

Per-core program (Tile framework):
  Phase 1: xw = x @ W + b for all 8192 tokens (S*16) as dense matmuls
           (M-tiles of 128 tokens), spilled to DRAM f32r; emission is
           interleaved into the step loop (lookahead 4 M-tiles) so its
           matmuls fill PE gaps instead of delaying early steps.
  Phase 2: 512 sequential LSTM steps. Per step, gate pre-activations
           accumulate in PSUM via f32r matmuls (lhsT = h^T [512,16] chunks,
           rhs = U [512, 2048]). Each gate's accumulation group is OPENED by
           an xw_t-inject matmul (lhsT = 16x16 identity, start=True) that
           depends only on the prefetched xw tile — the scheduler hoists
           these into the previous step's tail while PE waits for h. The
           K-chunks then run bank-major in order i, g, f, o so the chain-
           feeding gates close earliest; ACT applies sigmoid/tanh straight
           from PSUM as each bank closes, overlapping the rest of the
           stream. Activated gates are PE-transposed into a [128, 64] layout
           (13 ns each, back-to-back; o in a separate PSUM tile so algebra
           reads never false-serialize), where the whole c/h algebra runs
           with full 128-partition utilization — and h_new in that layout IS
           the next step's matmul lhsT, so there are no h transposes or
           copies on the recurrence critical path. hs is written to DRAM in
           the transposed layout; the host untransposes.

All matmul operands are float32r: full fp32 storage, PE runs at bf16 speed
for moving-dim >= 256, measured matmul error ~1.6e-4 relative (better than
tf32). End-to-end output error vs the fp32 reference lands ~1e-3.
"""

from contextlib import ExitStack

import numpy as np

import concourse.bacc as bacc
import concourse.bass as bass
import concourse.mybir as mybir
import concourse.tile as tile
from concourse.bass_utils import run_bass_kernel_spmd
from concourse.masks import make_identity

F32 = mybir.dt.float32
F32R = mybir.dt.float32r

S = 512
I = 512
H = 512
B = 64
NCORES = 8
BL = 16        # batch rows per core
G = 4 * H      # 2048 gate columns, order f|g|i|o
KC = 4         # contraction chunks of 128
NB = 4         # gate banks of 512 columns
SPT = 128 // BL  # steps per phase-1 M-tile (8)

GATE_FUNCS = [
    mybir.ActivationFunctionType.Sigmoid,  # f
    mybir.ActivationFunctionType.Tanh,     # g
    mybir.ActivationFunctionType.Sigmoid,  # i
    mybir.ActivationFunctionType.Sigmoid,  # o
]

# Set to True by test.py to collect a profile; exec time lands in LAST_EXEC_NS.
TRACE = False
LAST_EXEC_NS = None
LAST_RESULTS = None


def _build(n_steps):
    nc = bacc.Bacc("TRN2", target_bir_lowering=False, debug=False)
    ntok = n_steps * BL
    NT = ntok // 128  # phase-1 M-tiles

    xT = nc.declare_dram_parameter("xT", [I, ntok], F32R, isOutput=False)
    U = nc.declare_dram_parameter("U", [H, G], F32R, isOutput=False)
    W = nc.declare_dram_parameter("W", [I, G], F32R, isOutput=False)
    bv = nc.declare_dram_parameter("b", [G], F32, isOutput=False)
    hs = nc.declare_dram_parameter("hs", [n_steps, 128, KC * BL], F32R, isOutput=True)

    with tile.TileContext(nc) as tc, ExitStack() as ctx:
        const = ctx.enter_context(tc.tile_pool(name="const", bufs=1))
        U_sb = const.tile([128, KC, G], F32R, name="U_sb")
        nc.sync.dma_start(out=U_sb, in_=U.ap().rearrange("(k p) g -> p k g", p=128))
        W_sb = const.tile([128, KC, G], F32R, name="W_sb")
        nc.sync.dma_start(out=W_sb, in_=W.ap().rearrange("(k p) g -> p k g", p=128))
        b_sb = const.tile([128, G], F32, name="b_sb")
        bap = bv.ap()
        b_bcast = bass.AP(
            tensor=bap.tensor, offset=bap.offset, ap=[[0, 128], list(bap.ap[0])]
        )
        nc.gpsimd.dma_start(out=b_sb, in_=b_bcast)
        ident = const.tile([BL, BL], F32, name="ident")
        make_identity(nc, ident)
        identr = const.tile([BL, BL], F32R, name="identr")
        nc.vector.tensor_copy(identr, ident)

        dram = ctx.enter_context(tc.tile_pool(name="dram", bufs=1, space="DRAM"))
        xw_d = [
            dram.tile([128, G], F32R, tag=f"xw{m}", name=f"xw{m}") for m in range(NT)
        ]

        # ---------- phase 1: xw = x @ W + b for all tokens ----------
        p1l = ctx.enter_context(tc.tile_pool(name="p1l", bufs=3))
        st = ctx.enter_context(tc.tile_pool(name="st", bufs=2))
        xwp = ctx.enter_context(tc.tile_pool(name="xwp", bufs=6))
        gp = ctx.enter_context(tc.tile_pool(name="gp", bufs=5, space="PSUM"))
        smp = ctx.enter_context(tc.tile_pool(name="smp", bufs=1, space="PSUM"))
        tpp = ctx.enter_context(tc.tile_pool(name="tpp", bufs=1, space="PSUM"))
        wk = ctx.enter_context(tc.tile_pool(name="wk", bufs=2))
        p1o = ctx.enter_context(tc.tile_pool(name="p1o", bufs=2))
        xTr = xT.ap().rearrange("(k p) t -> p k t", p=128)

        def emit_p1_tile(m):
            lhs = p1l.tile([128, KC, 128], F32R, tag="lhs", name=f"lhs{m}")
            nc.sync.dma_start(out=lhs, in_=xTr[:, :, m * 128 : (m + 1) * 128])
            xwb = p1o.tile([128, G], F32R, tag="xwb", name=f"xwb{m}")
            for n in range(NB):
                nsl = slice(n * 512, (n + 1) * 512)
                ps = smp.tile([128, 512], F32, tag="mix", name=f"p1ps{m}_{n}")
                for k in range(KC):
                    nc.tensor.matmul(
                        ps,
                        lhsT=lhs[:, k, :],
                        rhs=W_sb[:, k, nsl],
                        start=(k == 0),
                        stop=(k == KC - 1),
                    )
                nc.vector.tensor_tensor(
                    xwb[:, nsl], ps, b_sb[:, nsl], mybir.AluOpType.add
                )
            nc.sync.dma_start(out=xw_d[m], in_=xwb)

        P1_LOOKAHEAD = 4
        for m in range(min(P1_LOOKAHEAD, NT)):
            emit_p1_tile(m)

        # ---------- phase 2: recurrence ----------

        hT = st.tile([128, KC, BL], F32R, tag="hT", name="hT_init")
        zsc = wk.tile([128, KC * BL], F32, tag="zsc", name="zsc")
        nc.vector.memset(zsc, 0.0)
        nc.vector.tensor_copy(hT, zsc)
        cT = st.tile([128, KC, BL], F32, tag="c", name="cT_init")
        nc.vector.memset(cT, 0.0)

        for t in range(n_steps):
            m, r = divmod(t, SPT)
            if r == 0 and m + P1_LOOKAHEAD < NT:
                emit_p1_tile(m + P1_LOOKAHEAD)
            xwt = xwp.tile([BL, G], F32R, tag="xwt", name=f"xwt{t}")
            nc.sync.dma_start(out=xwt, in_=xw_d[m][r * BL : (r + 1) * BL, :])

            gbs = []
            for n in range(NB):
                gbs.append(gp.tile([BL, 512], F32, tag="g", name=f"gates{t}_{n}"))
            acts = wk.tile([BL, G], F32, tag="acts", name=f"acts{t}")
            # transposed activations, 2 PSUM banks: f,g,i in bank 0 and o in
            # bank 1 (gate n lives at cols GOFF[n]:GOFF[n]+64), so the c/h
            # algebra's reads never serialize against o's late transposes.
            tpa = tpp.tile([128, 512], F32, tag="tpa", name=f"tpa{t}")
            tpo = tpp.tile([128, KC * BL], F32, tag="tpo", name=f"tpo{t}")
            GOFF = [0, 64, 128, 0]

            # xw-inject matmuls OPEN every accumulation group: they depend
            # only on the prefetched xw tile, so the scheduler hoists them
            # (and the p1 matmuls) into the previous step's tail while PE
            # would otherwise idle waiting for h.
            for n in (2, 0, 1, 3):
                nc.tensor.matmul(
                    gbs[n],
                    lhsT=identr,
                    rhs=xwt[:, n * 512 : (n + 1) * 512],
                    start=True,
                    stop=False,
                    skip_group_check=True,
                )
            # Bank-major K-chunks in order i, f, g, o: h arrives whole (one
            # hn write), so the chain-feeding banks can close as early as
            # possible and the tail chain (i/f/g acts -> gi/cf/cn -> tanh ->
            # hn) runs concurrently with the remaining o stream.
            act_insts = []
            for n in (2, 0, 1, 3):
                for k in range(KC):
                    nc.tensor.matmul(
                        gbs[n],
                        lhsT=hT[:, k, :],
                        rhs=U_sb[:, k, n * 512 : (n + 1) * 512],
                        start=False,
                        stop=(k == KC - 1),
                        skip_group_check=True,
                    )
                ai = nc.scalar.activation(
                    acts[:, n * 512 : (n + 1) * 512], gbs[n], GATE_FUNCS[n]
                )
                act_insts.append(ai.ins)
                dst = tpo if n == 3 else tpa
                for cc in range(KC):
                    nc.tensor.transpose(
                        dst[:, GOFF[n] + cc * BL : GOFF[n] + (cc + 1) * BL],
                        acts[:, n * 512 + cc * 128 : n * 512 + (cc + 1) * 128],
                        ident,
                    )
                if n == 2:
                    # bounce i^T to SBUF while the f,g streams run: gi then
                    # needs only ONE PSUM operand (g), removing a copy hop
                    # from the critical chain after g closes
                    isb = wk.tile([128, KC * BL], F32, tag="isb", name=f"isb{t}")
                    nc.vector.tensor_copy(isb, tpa[:, 128 : 128 + KC * BL])
                if n == 0:
                    cf = wk.tile([128, KC * BL], F32, tag="cf", name=f"cf{t}")
                    nc.vector.tensor_tensor(
                        cf, cT.rearrange("p c b -> p (c b)"), tpa[:, 0 : KC * BL],
                        mybir.AluOpType.mult,
                    )
                if n == 1:
                    gi = wk.tile([128, KC * BL], F32, tag="gi", name=f"gi{t}")
                    nc.vector.tensor_tensor(
                        gi, isb, tpa[:, 64 : 64 + KC * BL], mybir.AluOpType.mult
                    )
            cTn = st.tile([128, KC, BL], F32, tag="c", name=f"c{t}")
            tch = wk.tile([128, KC * BL], F32, tag="tch", name=f"tch{t}")
            hTn = st.tile([128, KC, BL], F32R, tag="hT", name=f"hT{t}")
            cTnf = cTn.rearrange("p c b -> p (c b)")
            hTnf = hTn.rearrange("p c b -> p (c b)")
            nc.vector.tensor_tensor(cTnf, cf, gi, mybir.AluOpType.add)
            nc.scalar.activation(tch, cTnf, mybir.ActivationFunctionType.Tanh)
            nc.vector.tensor_tensor(
                hTnf, tpo[:, 0 : KC * BL], tch, mybir.AluOpType.mult
            )
            nc.sync.dma_start(out=hs.ap()[t], in_=hTnf)
            hT = hTn
            cT = cTn

    nc.compile()
    return nc


_CACHE = {}


def _get_nc(n_steps):
    if n_steps not in _CACHE:
        _CACHE[n_steps] = _build(n_steps)
    return _CACHE[n_steps]


def kernel(x, mask, U_fwd, W_fwd, b_fwd, U_bwd, W_bwd, b_bwd, n_steps=None):
    global LAST_EXEC_NS, LAST_RESULTS
    x = np.asarray(x, dtype=np.float32)
    mask = np.asarray(mask)
    if n_steps is None:
        n_steps = x.shape[1]
    nc = _get_nc(n_steps)

    packs = {}
    for d, (U_, W_, b_) in (
        ("f", (U_fwd, W_fwd, b_fwd)),
        ("b", (U_bwd, W_bwd, b_bwd)),
    ):
        U_ = np.asarray(U_, dtype=np.float32)
        W_ = np.asarray(W_, dtype=np.float32)
        b_ = np.asarray(b_, dtype=np.float32)
        packs[d] = (
            np.ascontiguousarray(np.concatenate([U_[g] for g in range(4)], axis=1)),
            np.ascontiguousarray(np.concatenate([W_[g] for g in range(4)], axis=1)),
            np.ascontiguousarray(np.concatenate([b_[g] for g in range(4)])),
        )

    in_maps = []
    for c in range(NCORES):
        is_bwd = c >= 4
        q = c % 4
        xc = x[q * BL : (q + 1) * BL, :n_steps]
        if is_bwd:
            xc = xc[:, ::-1]
        Up, Wp, bp = packs["b" if is_bwd else "f"]
        xTc = np.ascontiguousarray(xc.transpose(2, 1, 0).reshape(I, n_steps * BL))
        in_maps.append({"xT": xTc, "U": Up, "W": Wp, "b": bp})

    r = run_bass_kernel_spmd(nc, in_maps, list(range(NCORES)), trace=TRACE)
    LAST_EXEC_NS = r.exec_time_ns
    LAST_RESULTS = r
    res = r.results

    def unpack(hs_t):
        # hs_t [S, 128, KC*BL] transposed-layout: hs_t[s, p, c*BL+b] = h[s, b, c*128+p]
        S_ = hs_t.shape[0]
        return hs_t.reshape(S_, 128, KC, BL).transpose(0, 3, 2, 1).reshape(S_, BL, H)

    fwd = np.concatenate([unpack(res[c]["hs"]) for c in range(4)], axis=1)
    bwd = np.concatenate([unpack(res[c]["hs"]) for c in range(4, 8)], axis=1)[::-1]
    output = np.ascontiguousarray(
        np.concatenate([fwd, bwd], axis=-1), dtype=np.float32
    )

    lengths = mask[:, :n_steps].sum(axis=1).astype(np.int32)
    fwd_idx = np.clip(lengths - 1, 0, None)
    bidx = np.arange(x.shape[0])
    h_f = np.ascontiguousarray(fwd[fwd_idx, bidx])
    h_b = np.ascontiguousarray(bwd[0, bidx])
    return output, h_f, h_b


# revision 43
# speedup vs baseline: 1.3700x; 1.0000x over previous
"""BiLSTM kernel for Trainium2 (8 NeuronCores, SPMD data-parallel).

Problem: x [B=64, S=512, I=512], BiLSTM with H=512, gate order (f, g, i, o).
Returns (output [S, B, 2H], h_f [B, H], h_b [B, H]) matching the reference.

Sharding: direction x batch. Cores 0-3 run the forward direction on batch
quarters (16 rows each); cores 4-7 run the backward direction on batch
quarters, fed time-reversed x (SPMD: all cores run the identical program,
only the data differs; host flips the backward outputs back).

Per-core program (Tile framework):
  Phase 1: xw = x @ W + b for all 8192 tokens (S*16) as dense matmuls
           (M-tiles of 128 tokens), spilled to DRAM f32r; emission is
           interleaved into the step loop (lookahead 4 M-tiles) so its
           matmuls fill PE gaps instead of delaying early steps.
  Phase 2: 512 sequential LSTM steps. Per step, gate pre-activations
           accumulate in PSUM via f32r matmuls (lhsT = h^T [512,16] chunks,
           rhs = U [512, 2048]). Each gate's accumulation group is OPENED by
           an xw_t-inject matmul (lhsT = 16x16 identity, start=True) that
           depends only on the prefetched xw tile — the scheduler hoists
           these into the previous step's tail while PE waits for h. The
           K-chunks then run bank-major in order i, g, f, o so the chain-
           feeding gates close earliest; ACT applies sigmoid/tanh straight
           from PSUM as each bank closes, overlapping the rest of the
           stream. Activated gates are PE-transposed into a [128, 64] layout
           (13 ns each, back-to-back; o in a separate PSUM tile so algebra
           reads never false-serialize), where the whole c/h algebra runs
           with full 128-partition utilization — and h_new in that layout IS
           the next step's matmul lhsT, so there are no h transposes or
           copies on the recurrence critical path. hs is written to DRAM in
           the transposed layout; the host untransposes.

All matmul operands are float32r: full fp32 storage, PE runs at bf16 speed
for moving-dim >= 256, measured matmul error ~1.6e-4 relative (better than
tf32). End-to-end output error vs the fp32 reference lands ~1e-3.
"""

from contextlib import ExitStack

import numpy as np

import concourse.bacc as bacc
import concourse.bass as bass
import concourse.mybir as mybir
import concourse.tile as tile
from concourse.bass_utils import run_bass_kernel_spmd
from concourse.masks import make_identity

F32 = mybir.dt.float32
F32R = mybir.dt.float32r

S = 512
I = 512
H = 512
B = 64
NCORES = 8
BL = 16        # batch rows per core
G = 4 * H      # 2048 gate columns, order f|g|i|o
KC = 4         # contraction chunks of 128
NB = 4         # gate banks of 512 columns
SPT = 128 // BL  # steps per phase-1 M-tile (8)

GATE_FUNCS = [
    mybir.ActivationFunctionType.Sigmoid,  # f
    mybir.ActivationFunctionType.Tanh,     # g
    mybir.ActivationFunctionType.Sigmoid,  # i
    mybir.ActivationFunctionType.Sigmoid,  # o
]

# Set to True by test.py to collect a profile; exec time lands in LAST_EXEC_NS.
TRACE = False
LAST_EXEC_NS = None
LAST_RESULTS = None


def _build(n_steps):
    nc = bacc.Bacc("TRN2", target_bir_lowering=False, debug=False)
    ntok = n_steps * BL
    NT = ntok // 128  # phase-1 M-tiles

    xT = nc.declare_dram_parameter("xT", [I, ntok], F32R, isOutput=False)
    U = nc.declare_dram_parameter("U", [H, G], F32R, isOutput=False)
    W = nc.declare_dram_parameter("W", [I, G], F32R, isOutput=False)
    bv = nc.declare_dram_parameter("b", [G], F32, isOutput=False)
    hs = nc.declare_dram_parameter("hs", [n_steps, 128, KC * BL], F32R, isOutput=True)

    with tile.TileContext(nc) as tc, ExitStack() as ctx:
        const = ctx.enter_context(tc.tile_pool(name="const", bufs=1))
        U_sb = const.tile([128, KC, G], F32R, name="U_sb")
        nc.sync.dma_start(out=U_sb, in_=U.ap().rearrange("(k p) g -> p k g", p=128))
        W_sb = const.tile([128, KC, G], F32R, name="W_sb")
        nc.sync.dma_start(out=W_sb, in_=W.ap().rearrange("(k p) g -> p k g", p=128))
        b_sb = const.tile([128, G], F32, name="b_sb")
        bap = bv.ap()
        b_bcast = bass.AP(
            tensor=bap.tensor, offset=bap.offset, ap=[[0, 128], list(bap.ap[0])]
        )
        nc.gpsimd.dma_start(out=b_sb, in_=b_bcast)
        ident = const.tile([BL, BL], F32, name="ident")
        make_identity(nc, ident)
        identr = const.tile([BL, BL], F32R, name="identr")
        nc.vector.tensor_copy(identr, ident)

        dram = ctx.enter_context(tc.tile_pool(name="dram", bufs=1, space="DRAM"))
        xw_d = [
            dram.tile([128, G], F32R, tag=f"xw{m}", name=f"xw{m}") for m in range(NT)
        ]

        # ---------- phase 1: xw = x @ W + b for all tokens ----------
        p1l = ctx.enter_context(tc.tile_pool(name="p1l", bufs=3))
        st = ctx.enter_context(tc.tile_pool(name="st", bufs=2))
        xwp = ctx.enter_context(tc.tile_pool(name="xwp", bufs=6))
        gp = ctx.enter_context(tc.tile_pool(name="gp", bufs=5, space="PSUM"))
        smp = ctx.enter_context(tc.tile_pool(name="smp", bufs=1, space="PSUM"))
        tpp = ctx.enter_context(tc.tile_pool(name="tpp", bufs=1, space="PSUM"))
        wk = ctx.enter_context(tc.tile_pool(name="wk", bufs=2))
        p1o = ctx.enter_context(tc.tile_pool(name="p1o", bufs=2))
        xTr = xT.ap().rearrange("(k p) t -> p k t", p=128)

        def emit_p1_tile(m):
            lhs = p1l.tile([128, KC, 128], F32R, tag="lhs", name=f"lhs{m}")
            nc.sync.dma_start(out=lhs, in_=xTr[:, :, m * 128 : (m + 1) * 128])
            xwb = p1o.tile([128, G], F32R, tag="xwb", name=f"xwb{m}")
            for n in range(NB):
                nsl = slice(n * 512, (n + 1) * 512)
                ps = smp.tile([128, 512], F32, tag="mix", name=f"p1ps{m}_{n}")
                for k in range(KC):
                    nc.tensor.matmul(
                        ps,
                        lhsT=lhs[:, k, :],
                        rhs=W_sb[:, k, nsl],
                        start=(k == 0),
                        stop=(k == KC - 1),
                    )
                nc.vector.tensor_tensor(
                    xwb[:, nsl], ps, b_sb[:, nsl], mybir.AluOpType.add
                )
            nc.sync.dma_start(out=xw_d[m], in_=xwb)

        P1_LOOKAHEAD = 4
        for m in range(min(P1_LOOKAHEAD, NT)):
            emit_p1_tile(m)

        # ---------- phase 2: recurrence ----------

        hT = st.tile([128, KC, BL], F32R, tag="hT", name="hT_init")
        zsc = wk.tile([128, KC * BL], F32, tag="zsc", name="zsc")
        nc.vector.memset(zsc, 0.0)
        nc.vector.tensor_copy(hT, zsc)
        cT = st.tile([128, KC, BL], F32, tag="c", name="cT_init")
        nc.vector.memset(cT, 0.0)

        for t in range(n_steps):
            m, r = divmod(t, SPT)
            xwt = xwp.tile([BL, G], F32R, tag="xwt", name=f"xwt{t}")
            nc.sync.dma_start(out=xwt, in_=xw_d[m][r * BL : (r + 1) * BL, :])

            gbs = []
            for n in range(NB):
                gbs.append(gp.tile([BL, 512], F32, tag="g", name=f"gates{t}_{n}"))
            acts = wk.tile([BL, G], F32, tag="acts", name=f"acts{t}")
            # transposed activations, 2 PSUM banks: f,g,i in bank 0 and o in
            # bank 1 (gate n lives at cols GOFF[n]:GOFF[n]+64), so the c/h
            # algebra's reads never serialize against o's late transposes.
            tpa = tpp.tile([128, 512], F32, tag="tpa", name=f"tpa{t}")
            tpo = tpp.tile([128, KC * BL], F32, tag="tpo", name=f"tpo{t}")
            GOFF = [0, 64, 128, 0]

            # xw-inject matmuls OPEN every accumulation group: they depend
            # only on the prefetched xw tile, so the scheduler hoists them
            # (and the p1 matmuls) into the previous step's tail while PE
            # would otherwise idle waiting for h.
            for n in (2, 0, 1, 3):
                nc.tensor.matmul(
                    gbs[n],
                    lhsT=identr,
                    rhs=xwt[:, n * 512 : (n + 1) * 512],
                    start=True,
                    stop=False,
                    skip_group_check=True,
                )
            # Bank-major K-chunks in order i, f, g, o: h arrives whole (one
            # hn write), so the chain-feeding banks can close as early as
            # possible and the tail chain (i/f/g acts -> gi/cf/cn -> tanh ->
            # hn) runs concurrently with the remaining o stream.
            act_insts = []
            for n in (2, 0, 1, 3):
                for k in range(KC):
                    nc.tensor.matmul(
                        gbs[n],
                        lhsT=hT[:, k, :],
                        rhs=U_sb[:, k, n * 512 : (n + 1) * 512],
                        start=False,
                        stop=(k == KC - 1),
                        skip_group_check=True,
                    )
                ai = nc.scalar.activation(
                    acts[:, n * 512 : (n + 1) * 512], gbs[n], GATE_FUNCS[n]
                )
                act_insts.append(ai.ins)
                dst = tpo if n == 3 else tpa
                for cc in range(KC):
                    nc.tensor.transpose(
                        dst[:, GOFF[n] + cc * BL : GOFF[n] + (cc + 1) * BL],
                        acts[:, n * 512 + cc * 128 : n * 512 + (cc + 1) * 128],
                        ident,
                    )
                if n == 2:
                    # bounce i^T to SBUF while the f,g streams run: gi then
                    # needs only ONE PSUM operand (g), removing a copy hop
                    # from the critical chain after g closes
                    isb = wk.tile([128, KC * BL], F32, tag="isb", name=f"isb{t}")
                    nc.vector.tensor_copy(isb, tpa[:, 128 : 128 + KC * BL])
                if n == 0:
                    cf = wk.tile([128, KC * BL], F32, tag="cf", name=f"cf{t}")
                    nc.vector.tensor_tensor(
                        cf, cT.rearrange("p c b -> p (c b)"), tpa[:, 0 : KC * BL],
                        mybir.AluOpType.mult,
                    )
                if n == 1:
                    gi = wk.tile([128, KC * BL], F32, tag="gi", name=f"gi{t}")
                    nc.vector.tensor_tensor(
                        gi, isb, tpa[:, 64 : 64 + KC * BL], mybir.AluOpType.mult
                    )
            cTn = st.tile([128, KC, BL], F32, tag="c", name=f"c{t}")
            tch = wk.tile([128, KC * BL], F32, tag="tch", name=f"tch{t}")
            hTn = st.tile([128, KC, BL], F32R, tag="hT", name=f"hT{t}")
            cTnf = cTn.rearrange("p c b -> p (c b)")
            hTnf = hTn.rearrange("p c b -> p (c b)")
            nc.vector.tensor_tensor(cTnf, cf, gi, mybir.AluOpType.add)
            nc.scalar.activation(tch, cTnf, mybir.ActivationFunctionType.Tanh)
            nc.vector.tensor_tensor(
                hTnf, tpo[:, 0 : KC * BL], tch, mybir.AluOpType.mult
            )
            nc.sync.dma_start(out=hs.ap()[t], in_=hTnf)
            hT = hTn
            cT = cTn
            # emitted after the step body: lower priority, so phase-1 matmuls
            # only fill PE gaps instead of preempting the critical stream
            if r == 0 and m + P1_LOOKAHEAD < NT:
                emit_p1_tile(m + P1_LOOKAHEAD)

    nc.compile()
    return nc


_CACHE = {}


def _get_nc(n_steps):
    if n_steps not in _CACHE:
        _CACHE[n_steps] = _build(n_steps)
    return _CACHE[n_steps]


def kernel(x, mask, U_fwd, W_fwd, b_fwd, U_bwd, W_bwd, b_bwd, n_steps=None):
    global LAST_EXEC_NS, LAST_RESULTS
    x = np.asarray(x, dtype=np.float32)
    mask = np.asarray(mask)
    if n_steps is None:
        n_steps = x.shape[1]
    nc = _get_nc(n_steps)

    packs = {}
    for d, (U_, W_, b_) in (
        ("f", (U_fwd, W_fwd, b_fwd)),
        ("b", (U_bwd, W_bwd, b_bwd)),
    ):
        U_ = np.asarray(U_, dtype=np.float32)
        W_ = np.asarray(W_, dtype=np.float32)
        b_ = np.asarray(b_, dtype=np.float32)
        packs[d] = (
            np.ascontiguousarray(np.concatenate([U_[g] for g in range(4)], axis=1)),
            np.ascontiguousarray(np.concatenate([W_[g] for g in range(4)], axis=1)),
            np.ascontiguousarray(np.concatenate([b_[g] for g in range(4)])),
        )

    in_maps = []
    for c in range(NCORES):
        is_bwd = c >= 4
        q = c % 4
        xc = x[q * BL : (q + 1) * BL, :n_steps]
        if is_bwd:
            xc = xc[:, ::-1]
        Up, Wp, bp = packs["b" if is_bwd else "f"]
        xTc = np.ascontiguousarray(xc.transpose(2, 1, 0).reshape(I, n_steps * BL))
        in_maps.append({"xT": xTc, "U": Up, "W": Wp, "b": bp})

    r = run_bass_kernel_spmd(nc, in_maps, list(range(NCORES)), trace=TRACE)
    LAST_EXEC_NS = r.exec_time_ns
    LAST_RESULTS = r
    res = r.results

    def unpack(hs_t):
        # hs_t [S, 128, KC*BL] transposed-layout: hs_t[s, p, c*BL+b] = h[s, b, c*128+p]
        S_ = hs_t.shape[0]
        return hs_t.reshape(S_, 128, KC, BL).transpose(0, 3, 2, 1).reshape(S_, BL, H)

    fwd = np.concatenate([unpack(res[c]["hs"]) for c in range(4)], axis=1)
    bwd = np.concatenate([unpack(res[c]["hs"]) for c in range(4, 8)], axis=1)[::-1]
    output = np.ascontiguousarray(
        np.concatenate([fwd, bwd], axis=-1), dtype=np.float32
    )

    lengths = mask[:, :n_steps].sum(axis=1).astype(np.int32)
    fwd_idx = np.clip(lengths - 1, 0, None)
    bidx = np.arange(x.shape[0])
    h_f = np.ascontiguousarray(fwd[fwd_idx, bidx])
    h_b = np.ascontiguousarray(bwd[0, bidx])
    return output, h_f, h_b
